# revision 2
# baseline (speedup 1.0000x reference)
"""CrossCoderDecoder forward on 8 trn2 NeuronCores.

x[b,l,d] = sum_f f[b,f] * weight[l,f,d] + bias[l,d]
B=32, L=2, F=65536, D=768, fp32.

Sharding: the F (dict) axis is split 8 ways (8192 features per core).
Each core computes its partial [L, 2*B, D] sums; the host sums the 8
partials (and the hi/lo half-pair, see below) and adds the bias (the
"all-reduce" of the sharding hint, done host-side since the output is
tiny).

Precision/perf scheme: the kernel is HBM-bandwidth bound (each core
must read its 50 MB weight shard), so the weights are streamed as
float8e3 (E3M4: 1+3+4, bias 3) -- 1 byte/element, 4x fewer bytes than
fp32, 2x fewer than the bf16 hi/lo scheme.  E3M4 keeps 4 mantissa
bits; with the weight pre-scaled by 256 (sigma_w = 1/256 -> ~1) the
whole Gaussian weight distribution sits in E3M4's normal range and
the end-to-end output error is ~1.3e-2 relative (gate: 2e-2),
dominated entirely by the weight quantization.

f stays near-fp32: the PE stationary operand packs fh = e3m4(f) and
fl = e3m4(64*(f - fh)) side by side ([128, 64] lhsT), so one
streaming pass of the weights computes both the hi and lo partial
products into psum[0:32] / psum[32:64].  The host combines
(hi + lo/64)/256 during the partial reduction, making the f-side
quantization error negligible (~1e-4).

Weight DMA layout: per (l, chunk of CHUNK_ROWS k-rows) one dma_start
moves a contiguous [P, KO, D] fp8 block into SBUF such that each
partition reads one contiguous line (KO*D bytes).  The host pre-packs
the weights into exactly that image and pre-permutes f into
fhl[p, j, 64] with the matching k order (k = ch*CHUNK_ROWS + p*KO + o
at j = ch*KO + o).
"""

import numpy as np
import ml_dtypes

import concourse.bass as bass
import concourse.tile as tile
from concourse import bacc, mybir
from concourse import bass_utils

B, L, F, D = 32, 2, 65536, 768
NCORES = 8
FS = F // NCORES          # 8192 features per core
P = 128
CHUNK_ROWS = 1024         # k-rows per weight DMA (768 KB fp8 chunks)
CH = FS // CHUNK_ROWS     # chunks per l
KO = CHUNK_ROWS // P      # k-subtiles per chunk
W_BUFS = 12               # weight tile double-buffering depth
W_SINGLE_RING = False     # True: all w DMAs on the SP ring (slower)
NSPLITS = ((0, 512), (512, 768))  # PSUM-bank splits of D
WSCALE = 256.0            # weight pre-scale (exact power of 2)
FLSCALE = 64.0            # f-lo pre-scale (exact power of 2)

_F32 = mybir.dt.float32
_FP8 = mybir.dt.float8e3
_FP8_NP = ml_dtypes.float8_e3m4

_cache = {}


def set_tiling(chunk_rows: int, w_bufs: int | None = None):
    """Adjust chunking (for tuning sweeps); drops the cached program."""
    global CHUNK_ROWS, CH, KO, W_BUFS
    CHUNK_ROWS = chunk_rows
    CH = FS // CHUNK_ROWS
    KO = CHUNK_ROWS // P
    if w_bufs is not None:
        W_BUFS = w_bufs
    _cache.clear()


def _build():
    """Build + schedule the (per-core identical) Bass program once."""
    nc = bacc.Bacc("TRN2", target_bir_lowering=False, debug=False)

    fhl = nc.dram_tensor("fhl", [P, CH * KO, 2 * B], _FP8, kind="ExternalInput").ap()
    w = nc.dram_tensor("w", [L, CH, P, KO, D], _FP8, kind="ExternalInput").ap()
    out = nc.dram_tensor("out", [L, 2 * B, D], _F32, kind="ExternalOutput").ap()

    with tile.TileContext(nc) as tc:
        with (
            tc.tile_pool(name="fpool", bufs=1) as fpool,
            tc.tile_pool(name="wpool", bufs=W_BUFS) as wpool,
            tc.tile_pool(name="opool", bufs=2) as opool,
            tc.tile_pool(name="psum", bufs=1, space="PSUM") as psum,
        ):
            # fhl rides the ACT HWDGE ring so it overlaps the first w
            # chunks (the SP ring is FIFO per issuing engine).
            f_sb = fpool.tile([P, CH * KO, 2 * B], _FP8)
            nc.scalar.dma_start(f_sb[:], fhl[:])

            # Both l-groups' PSUM accumulators stay open for the whole
            # kernel; chunks interleave l so the DMA stream never hits a
            # drain point until the very end.
            ps = [
                [
                    psum.tile([2 * B, n1 - n0], _F32, name=f"ps_{l}_{i}")
                    for i, (n0, n1) in enumerate(NSPLITS)
                ]
                for l in range(L)
            ]
            for ch in range(CH):
                for l in range(L):
                    wt = wpool.tile([P, KO, D], _FP8)
                    dma_eng = (
                        nc.sync
                        if (W_SINGLE_RING or (ch * L + l) % 2 == 0)
                        else nc.scalar
                    )
                    dma_eng.dma_start(wt[:], w[l, ch])
                    for o in range(KO):
                        j = ch * KO + o
                        first = j == 0
                        last = j == CH * KO - 1
                        for i, (n0, n1) in enumerate(NSPLITS):
                            nc.tensor.matmul(
                                ps[l][i][:],
                                f_sb[:, j, :],
                                wt[:, o, n0:n1],
                                start=first,
                                stop=last,
                            )
            for l in range(L):
                out_sb = opool.tile([2 * B, D], _F32)
                for i, (n0, n1) in enumerate(NSPLITS):
                    nc.vector.tensor_copy(out=out_sb[:, n0:n1], in_=ps[l][i][:])
                nc.scalar.dma_start(out[l], out_sb[:])

    nc.compile()
    return nc


def _prep_f(f_core: np.ndarray) -> np.ndarray:
    """f_core [B, FS] -> fhl [P, CH*KO, 2*B] e3m4 matching the kernel's
    k order (k = ch*CHUNK_ROWS + p*KO + o at fhl[p, ch*KO + o]); the
    last axis holds fh[b] in [0, B) and fl[b]*FLSCALE in [B, 2B)."""
    hi = f_core.astype(_FP8_NP)
    lo = ((f_core - hi.astype(np.float32)) * FLSCALE).astype(_FP8_NP)
    ft = np.concatenate([hi.T, lo.T], axis=1)          # [FS, 2B]
    ft = ft.reshape(CH, P, KO, 2 * B).transpose(1, 0, 2, 3)
    return np.ascontiguousarray(ft.reshape(P, CH * KO, 2 * B))


def _prep_w(w_core: np.ndarray) -> np.ndarray:
    """w_core [L, FS, D] -> [L, CH, P, KO, D] e3m4 (exact SBUF image),
    pre-scaled by WSCALE so sigma lands in E3M4's normal range."""
    wq = (w_core * WSCALE).astype(_FP8_NP)
    return np.ascontiguousarray(wq.reshape(L, CH, P, KO, D))


def kernel(f: np.ndarray, weight: np.ndarray, bias: np.ndarray) -> np.ndarray:
    f = np.asarray(f, dtype=np.float32)
    weight = np.asarray(weight, dtype=np.float32)
    bias = np.asarray(bias, dtype=np.float32)

    if "nc" not in _cache:
        _cache["nc"] = _build()
    nc = _cache["nc"]

    in_maps = []
    for c in range(NCORES):
        sl = slice(c * FS, (c + 1) * FS)
        in_maps.append(
            {
                "fhl": _prep_f(f[:, sl]),
                "w": _prep_w(weight[:, sl, :]),
            }
        )

    res = bass_utils.run_bass_kernel_spmd(nc, in_maps, core_ids=list(range(NCORES)))
    partial = np.stack([r["out"] for r in res.results])  # [NCORES, L, 2B, D]
    total = partial.sum(axis=0)                          # [L, 2B, D]
    total = total[:, :B, :] + total[:, B:, :] / FLSCALE  # hi + lo/FLSCALE
    x = total.transpose(1, 0, 2) / WSCALE + bias[None, :, :]  # [B, L, D]
    return x.astype(np.float32)
